# revision 1
# baseline (speedup 1.0000x reference)
"""ExternalAttention Trainium2 Bass kernel.

Math (per batch b, with N = H*W = 4096 tokens, C = 512, K = 64):
    x      = inputs @ w1 + b1          [N, C]
    logits = x @ m0                    [N, K]
    attn   = softmax(logits, axis=N)   (the following L1-normalize over N is a
                                        divide by 1 + 1e-9 -> skipped; the max
                                        subtraction is shift-invariant and
                                        logits are O(1) -> skipped)
    y      = attn @ m1 @ w2            [N, C]
    out    = relu(BN_affine(y) + inputs)

Host-side folds (all tiny C x C / C x K matrices):
    wm    = [w1 @ m0 | 0]                           [C, K+1]  (b1 @ m0 shifts each
            softmax column by a constant -> softmax-invariant, dropped; the zero
            column makes exp produce a ones-row that injects the BN shift)
    scale = gamma / sqrt(bn_var + eps); shift = beta - bn_mean * scale
    w2m   = [m1 @ (w2 * scale) ; shift]             [K+1, C]
    => out = relu(colsoftmax(inputs @ wm_aug) @ w2m_aug + inputs)

Device kernel (per core, 2 batches, data-parallel over B=16 on 8 cores).
Matmul operands are float32r (fp32 storage, full PE rate); inputs, residual
and outputs stay exact fp32. Tokens are interleaved n = base + p*4 + e so
each DMA descriptor moves 8KB contiguous per partition.
    - load A tiles [128, 2, 4, 512] (1MB DMAs on the sync ring)
    - PE-transpose A into A^T psum chunks, copy to SBUF (ACT/DVE, rounds to
      f32r), mm1 -> logitsT chunk [65, 512] in psum, software-pipelined one
      chunk behind the transposes
    - ACT exp straight from psum into attn [65, N] with accumulated row sums;
      DVE reciprocal + per-row scale (column softmax; max-shift skipped)
    - mm2 per 256-token super-tile: psum = attn_aug^T @ w2m_aug (shift via
      ones row), DVE adds the exact fp32 residual, ACT relu -> SBUF, store
    - two-batch software pipeline: batch 1 chunks interleave with batch 0's
      epilogue so the in-order PE stream never waits on a softmax
"""

import os
import sys
from contextlib import ExitStack

import numpy as np

for _p in ("/opt/trn_rl_repo", os.path.expanduser("~/.axon_site/_ro/trn_rl_repo")):
    if os.path.isdir(_p) and _p not in sys.path:
        sys.path.insert(0, _p)

import concourse.bass as bass
import concourse.mybir as mybir
import concourse.tile as tile
from concourse import bacc
from concourse.bass import ts
from concourse.bass_utils import run_bass_kernel_spmd

B, H, W, C, K = 16, 64, 64, 512, 64
N = H * W  # 4096 tokens
BN_EPS = 1e-3
NCORES = 8
BPC = B // NCORES  # batches per core = 2

F32 = mybir.dt.float32
F32R = mybir.dt.float32r

NG = 4               # token groups of 1024 per batch
E = 4                # tokens interleaved per partition (8KB DMA runs)
NCHUNK = N // 512    # 8 n-chunks of 512 per batch; chunk q = (g, t)

_cached_nc = None


def _build_nc() -> bass.Bass:
    nc = bacc.Bacc(None, target_bir_lowering=False, debug=False)
    x = nc.dram_tensor("x", [BPC, N, C], F32, kind="ExternalInput")
    wm = nc.dram_tensor("wm", [C, K + 1], F32R, kind="ExternalInput")
    w2m = nc.dram_tensor("w2m", [K + 1, C], F32R, kind="ExternalInput")
    ident = nc.dram_tensor("ident", [128, 128], F32, kind="ExternalInput")
    y = nc.dram_tensor("y", [BPC, N, C], F32, kind="ExternalOutput")

    with tile.TileContext(nc) as tc, ExitStack() as ctx:
        const = ctx.enter_context(tc.tile_pool(name="const", bufs=1))
        a_pool = ctx.enter_context(tc.tile_pool(name="a", bufs=2 * NG))
        at_pool = ctx.enter_context(tc.tile_pool(name="at", bufs=2))
        attn_pool = ctx.enter_context(tc.tile_pool(name="attn", bufs=2))
        small = ctx.enter_context(tc.tile_pool(name="small", bufs=4))

        xbs, ybs = [], []
        for b in range(BPC):
            # token n = g*1024 + t*512 + p*4 + e
            xbs.append(x[b].rearrange("(g t p e) c -> g p t e c", g=NG, t=2, p=128))
            ybs.append(y[b].rearrange("(g t p e) c -> g p t e c", g=NG, t=2, p=128))

        a_bigs, attns, sums_t, at_tiles, p_ls = [], [], [], {}, {}

        def load_batch(b, first=False):
            a_big = []
            for g in range(NG):
                ag = a_pool.tile([128, 2, E, C], F32, tag="a")
                for t in range(2):
                    if first and g == 0 and t == 0:
                        nc.sync.dma_start(out=ag[:, 0, 0:2],
                                          in_=xbs[b][g][:, 0, 0:2])
                        nc.sync.dma_start(out=ag[:, 0, 2:4],
                                          in_=xbs[b][g][:, 0, 2:4])
                    else:
                        nc.sync.dma_start(out=ag[:, t], in_=xbs[b][g][:, t])
                    if first and g == 0 and t == 0:
                        # constants ride behind the very first token tile
                        nc.sync.dma_start(out=ident_sb, in_=ident[:, :])
                        nc.sync.dma_start(
                            out=wm_sb,
                            in_=wm.rearrange("(c4 p) k -> p c4 k", p=128))
                        nc.sync.dma_start(out=w2m_sb, in_=w2m[:, :])
                a_big.append(ag)
            a_bigs.append(a_big)
            attn = attn_pool.tile([K + 1, N], F32R, tag="attn", name=f"attn{b}")
            sums = small.tile([K + 1, NCHUNK], F32, tag="sums", name=f"sums{b}")
            attns.append(attn)
            sums_t.append(sums)

        def tr_part(tr_psum, b, q):
            """PE-transpose one 512-token chunk into SBUF A^T staging."""
            a_big = a_bigs[b]
            g, t = divmod(q, 2)
            at_tile = at_pool.tile([128, 4, 512], F32R, tag="at",
                                   name=f"at{b}_{q}")
            at_tiles[(b, q)] = at_tile
            for c4 in range(4):
                p_tr = tr_psum.tile([128, 512], F32, tag="tr")
                for e in range(E):
                    nc.tensor.transpose(
                        p_tr[:, ts(e, 128)],
                        a_big[g][:, t, e, ts(c4, 128)],
                        ident_sb,
                    )
                if c4 % 2 == 0:
                    nc.scalar.copy(at_tile[:, c4], p_tr)
                else:
                    nc.vector.tensor_copy(at_tile[:, c4], p_tr)

        def mm1_part(l_psum, b, q):
            """mm1 + exp for a chunk transposed earlier."""
            attn, sums = attns[b], sums_t[b]
            at_tile = at_tiles[(b, q)]
            p_l = l_psum.tile([K + 1, 512], F32, tag="l")
            for c4 in range(4):
                nc.tensor.matmul(
                    p_l,
                    lhsT=wm_sb[:, c4],
                    rhs=at_tile[:, c4],
                    start=(c4 == 0),
                    stop=(c4 == 3),
                )
            # exp straight from psum; row K is exp(0)=1 (ones row);
            # per-chunk row sums accumulate into sums[:, q]
            nc.scalar.activation(
                out=attn[:, ts(q, 512)], in_=p_l,
                func=mybir.ActivationFunctionType.Exp,
                accum_out=sums[:, q:q + 1],
            )

        def softmax_finish(b):
            attn, sums = attns[b], sums_t[b]
            total = small.tile([K + 1, 1], F32, tag="total")
            nc.vector.reduce_sum(out=total, in_=sums, axis=mybir.AxisListType.X)
            rsum = small.tile([K + 1, 1], F32, tag="rsum")
            nc.vector.reciprocal(out=rsum, in_=total)
            nc.vector.tensor_scalar_mul(attn[0:K], attn[0:K], rsum[0:K])

        def mm2(y_psum, b, lo, hi):
            """attn @ w2m + residual + relu over super-tiles [lo, hi)."""
            attn, a_big = attns[b], a_bigs[b]
            for st in range(lo, hi):
                gt, half = divmod(st, 2)
                g, t = divmod(gt, 2)
                sub = half * 2
                nt = gt * E + sub
                p_y = y_psum.tile([128, 2, C], F32, tag="y")
                for j in range(2):
                    nc.tensor.matmul(
                        p_y[:, j],
                        lhsT=attn[:, ts(nt + j, 128)],
                        rhs=w2m_sb,
                        start=True, stop=True,
                    )
                nc.vector.tensor_add(p_y, p_y, a_big[g][:, t, sub:sub + 2])
                nc.scalar.activation(
                    out=a_big[g][:, t, sub:sub + 2], in_=p_y,
                    func=mybir.ActivationFunctionType.Relu,
                )
                if half == 1:
                    nc.gpsimd.dma_start(out=ybs[b][g][:, t], in_=a_big[g][:, t])

        ident_sb = const.tile([128, 128], F32)
        wm_sb = const.tile([128, 4, K + 1], F32R)  # [p, c4, k] = wm[c4*128+p, k]
        w2m_sb = const.tile([K + 1, C], F32R)

        load_batch(0, first=True)
        load_batch(1)

        with tc.tile_pool(name="trps", bufs=2, space="PSUM") as tr_psum, \
             tc.tile_pool(name="lps", bufs=2, space="PSUM") as l_psum, \
             tc.tile_pool(name="yps", bufs=2, space="PSUM") as y_psum:
            # phase 1 for b0 with mm1 software-pipelined one chunk behind
            for q in range(NCHUNK):
                tr_part(tr_psum, 0, q)
                if q:
                    mm1_part(l_psum, 0, q - 1)
            mm1_part(l_psum, 0, NCHUNK - 1)
            softmax_finish(0)
            # b1 chunks interleaved with b0's epilogue (12 of 16 super-tiles)
            for q in range(NCHUNK):
                tr_part(tr_psum, 1, q)
                if q:
                    mm1_part(l_psum, 1, q - 1)
                if q < 6:
                    mm2(y_psum, 0, 2 * q, 2 * q + 2)
            mm1_part(l_psum, 1, NCHUNK - 1)
            # b1 softmax chain (DVE) runs while the PE chews the remaining
            # b0 epilogue tiles
            softmax_finish(1)
            mm2(y_psum, 0, 12, 16)

        # phase-1 psum banks are free now: deeper pipeline for b1's epilogue
        with tc.tile_pool(name="yps2", bufs=4, space="PSUM") as y_psum2:
            mm2(y_psum2, 1, 0, 16)

    nc.finalize()
    return nc


def _get_nc() -> bass.Bass:
    global _cached_nc
    if _cached_nc is None:
        _cached_nc = _build_nc()
    return _cached_nc


def _fold_weights(w1, m0, m1, w2, gamma, beta, bn_mean, bn_var):
    w1 = np.asarray(w1, np.float64)
    m0 = np.asarray(m0, np.float64)
    m1 = np.asarray(m1, np.float64)
    w2 = np.asarray(w2, np.float64)
    gamma = np.asarray(gamma, np.float64)
    beta = np.asarray(beta, np.float64)
    bn_mean = np.asarray(bn_mean, np.float64)
    bn_var = np.asarray(bn_var, np.float64)

    wm_aug = np.zeros((C, K + 1), np.float32)
    wm_aug[:, :K] = (w1 @ m0).astype(np.float32)  # col K stays 0 -> ones row
    scale = gamma / np.sqrt(bn_var + BN_EPS)
    w2m_aug = np.zeros((K + 1, C), np.float32)
    w2m_aug[:K] = (m1 @ (w2 * scale[None, :])).astype(np.float32)
    w2m_aug[K] = (beta - bn_mean * scale).astype(np.float32)  # shift row
    return wm_aug, w2m_aug


def _run(inputs_np: dict, trace: bool = False):
    nc = _get_nc()
    inp = np.ascontiguousarray(np.asarray(inputs_np["inputs"], np.float32))
    wm_aug, w2m_aug = _fold_weights(
        inputs_np["w1"], inputs_np["m0"], inputs_np["m1"], inputs_np["w2"],
        inputs_np["gamma"], inputs_np["beta"],
        inputs_np["bn_mean"], inputs_np["bn_var"],
    )
    eye = np.eye(128, dtype=np.float32)
    flat = inp.reshape(B, N, C)
    in_maps = [
        {
            "x": np.ascontiguousarray(flat[i * BPC:(i + 1) * BPC]),
            "wm": wm_aug,
            "w2m": w2m_aug,
            "ident": eye,
        }
        for i in range(NCORES)
    ]
    res = run_bass_kernel_spmd(nc, in_maps, core_ids=list(range(NCORES)), trace=trace)
    out = np.concatenate([r["y"] for r in res.results], axis=0)
    return out.reshape(B, H, W, C), res


def kernel(**inputs) -> np.ndarray:
    out, _ = _run(inputs, trace=False)
    return out



# revision 2
# speedup vs baseline: 1.2564x; 1.2564x over previous
"""ExternalAttention Trainium2 Bass kernel (bf16 I/O, transposed layout).

Math (per batch b, with N = H*W = 4096 tokens, C = 512, K = 64):
    x      = inputs @ w1 + b1          [N, C]
    logits = x @ m0                    [N, K]
    attn   = softmax(logits, axis=N)   (the L1-normalize over N afterwards is a
                                        divide by 1 + 1e-9 -> skipped; the max
                                        subtraction is shift-invariant and
                                        logits are O(1) -> skipped)
    y      = attn @ m1 @ w2            [N, C]
    out    = relu(BN_affine(y) + inputs)

Host-side folds (all tiny C x C / C x K matrices):
    wm    = [w1 @ m0 | 0]                           [C, K+1]  (b1 @ m0 shifts each
            softmax column by a constant -> softmax-invariant, dropped; the zero
            column makes exp produce a ones-row that injects the BN shift)
    scale = gamma / sqrt(bn_var + eps); shift = beta - bn_mean * scale
    w2m   = [m1 @ (w2 * scale) ; shift]             [K+1, C]
    => out = relu(colsoftmax(inputs @ wm_aug) @ w2m_aug + inputs)

The kernel is HBM-bandwidth-bound, so everything is stored bf16 (rel-err
budget 2e-2, bf16 contributes ~4e-3) and the host pre-transposes inputs to
x^T [C, N] per batch so no PE transposes / psum copies are needed on device.

Device kernel (per core, 2 batches, data-parallel over B=16 on 8 cores):
    - DMA x^T c4-row-block tiles [128, N] bf16 straight from HBM (contiguous)
    - mm1: logitsT chunk [65, 512] psum = sum_c4 wm[c4]^T @ xT[c4][:, chunk]
    - ACT exp psum -> attn [65, N] bf16 with accumulated row sums
    - softmax normalization folded into w2m: w2m_s = w2m * (1/total) per row
      (65 x 512 scale instead of 65 x 4096 -> DVE nearly free)
    - mm2 per chunk: psum[c4] = w2m_s[c4-slice]^T @ attn + I^T @ xT[c4]
      (residual injected on the PE via identity-matmul psum accumulation)
    - relu psum -> bf16 back into the xT tiles (ACT for c4 0-1, DVE for 2-3),
      stores per half-row-block on the ACT HWDGE ring / gpsimd SWDGE ring
    - ~10 warmup matmuls at t0 to lift the PE HAM clock gate 1.2 -> 2.4 GHz
      while the first DMAs land
"""

import os
import sys
from contextlib import ExitStack

import numpy as np
import ml_dtypes

for _p in ("/opt/trn_rl_repo", os.path.expanduser("~/.axon_site/_ro/trn_rl_repo")):
    if os.path.isdir(_p) and _p not in sys.path:
        sys.path.insert(0, _p)

import concourse.bass as bass
import concourse.mybir as mybir
import concourse.tile as tile
from concourse import bacc
from concourse.bass import ts
from concourse.bass_utils import run_bass_kernel_spmd

B, H, W, C, K = 16, 64, 64, 512, 64
N = H * W  # 4096 tokens
BN_EPS = 1e-3
NCORES = 8
BPC = B // NCORES  # batches per core = 2
NCHUNK = 8  # 512-token chunks per batch
NQ = 4  # DMA load quarters (1024 columns each)

F32 = mybir.dt.float32
BF16 = mybir.dt.bfloat16
NPBF16 = ml_dtypes.bfloat16

_cached_nc = None


def _build_nc() -> bass.Bass:
    nc = bacc.Bacc(None, target_bir_lowering=False, debug=False)
    x = nc.dram_tensor("x", [BPC, C, N], BF16, kind="ExternalInput")
    wm = nc.dram_tensor("wm", [128, 4 * (K + 1)], BF16, kind="ExternalInput")
    w2m = nc.dram_tensor("w2m", [K + 1, C], BF16, kind="ExternalInput")
    ident = nc.dram_tensor("ident", [128, 128], BF16, kind="ExternalInput")
    y = nc.dram_tensor("y", [BPC, C, N], BF16, kind="ExternalOutput")

    with tile.TileContext(nc) as tc, ExitStack() as ctx:
        const = ctx.enter_context(tc.tile_pool(name="const", bufs=1))
        xt_pool = ctx.enter_context(tc.tile_pool(name="xt", bufs=2 * 4))
        attn_pool = ctx.enter_context(tc.tile_pool(name="attn", bufs=2))
        small = ctx.enter_context(tc.tile_pool(name="small", bufs=2))

        ident_sb = const.tile([128, 128], BF16)
        wm_sb = const.tile([128, 4, K + 1], BF16)  # [p, c4, k] = wm[c4*128+p, k]
        w2m_sb = const.tile([K + 1, C], BF16)
        warm_sb = const.tile([128, 512], BF16)

        # constants lead the SP load ring
        nc.sync.dma_start(out=ident_sb, in_=ident[:, :])
        nc.sync.dma_start(
            out=wm_sb, in_=wm.rearrange("p (c4 k) -> p c4 k", c4=4))
        nc.sync.dma_start(out=w2m_sb, in_=w2m[:, :])
        # warmup rhs on the otherwise-idle gpsimd engine
        nc.gpsimd.memset(warm_sb, 0.0)

        xv = [x[b].rearrange("(c4 p) n -> c4 p n", p=128) for b in range(BPC)]
        yv = [y[b].rearrange("(c4 p) n -> c4 p n", p=128) for b in range(BPC)]

        xts, attns, sums_t, w2ms = [], [], [], []
        for b in range(BPC):
            xts.append([
                xt_pool.tile([128, N], BF16, tag="xt", name=f"xt{b}_{c4}")
                for c4 in range(4)
            ])
            attns.append(attn_pool.tile([K + 1, N], BF16, tag="attn",
                                        name=f"attn{b}"))
            sums_t.append(small.tile([K + 1, NCHUNK], F32, tag="sums",
                                     name=f"sums{b}"))
            w2ms.append(small.tile([K + 1, C], BF16, tag="w2ms",
                                   name=f"w2ms{b}"))

        # all input loads issued up front: quarter-column granularity so mm1
        # can start ~3us in; SP HWDGE ring drains them back-to-back
        for b in range(BPC):
            for quar in range(NQ):
                cs = ts(quar, N // NQ)
                for c4 in range(4):
                    nc.sync.dma_start(out=xts[b][c4][:, cs],
                                      in_=xv[b][c4][:, cs])

        def mm1(l_psum, b, q):
            p_l = l_psum.tile([K + 1, 512], F32, tag="l")
            for c4 in range(4):
                nc.tensor.matmul(
                    p_l,
                    lhsT=wm_sb[:, c4],
                    rhs=xts[b][c4][:, ts(q, 512)],
                    start=(c4 == 0),
                    stop=(c4 == 3),
                )
            # exp straight from psum; row K is exp(0)=1 (ones row);
            # per-chunk row sums accumulate into sums[:, q]
            nc.scalar.activation(
                out=attns[b][:, ts(q, 512)], in_=p_l,
                func=mybir.ActivationFunctionType.Exp,
                accum_out=sums_t[b][:, q:q + 1],
            )

        def softmax_fin(b):
            # column-softmax normalization folded into w2m rows (k-contraction)
            total = small.tile([K + 1, 1], F32, tag="tot")
            nc.vector.reduce_sum(out=total, in_=sums_t[b],
                                 axis=mybir.AxisListType.X)
            rsum = small.tile([K + 1, 1], F32, tag="rs")
            nc.vector.reciprocal(out=rsum, in_=total)
            nc.vector.tensor_scalar_mul(w2ms[b][0:K], w2m_sb[0:K], rsum[0:K])
            # shift row (ones row of attn) stays unscaled
            nc.vector.tensor_copy(w2ms[b][K:K + 1], w2m_sb[K:K + 1])

        def mm2(y_psum, b, q):
            qs = ts(q, 512)
            for c4 in range(4):
                p_y = y_psum.tile([128, 512], F32, tag="y")
                nc.tensor.matmul(p_y, lhsT=w2ms[b][:, ts(c4, 128)],
                                 rhs=attns[b][:, qs], start=True, stop=False)
                # residual: psum += I^T @ xT chunk
                nc.tensor.matmul(p_y, lhsT=ident_sb, rhs=xts[b][c4][:, qs],
                                 start=False, stop=True)
                if c4 < 2:
                    nc.scalar.activation(
                        out=xts[b][c4][:, qs], in_=p_y,
                        func=mybir.ActivationFunctionType.Relu)
                else:
                    nc.vector.tensor_scalar_max(xts[b][c4][:, qs], p_y, 0.0)
            if q % 4 == 3:
                hs = ts(q // 4, N // 2)
                for c4 in range(4):
                    eng = nc.scalar if c4 < 2 else nc.gpsimd
                    eng.dma_start(out=yv[b][c4][:, hs], in_=xts[b][c4][:, hs])

        with tc.tile_pool(name="lps", bufs=2, space="PSUM") as l_psum, \
             tc.tile_pool(name="yps", bufs=6, space="PSUM") as y_psum:
            # PE warmup: lift the HAM clock gate while the first loads land
            wp = y_psum.tile([128, 512], F32, tag="y", name="warm")
            for i in range(10):
                nc.tensor.matmul(wp, lhsT=ident_sb, rhs=warm_sb,
                                 start=(i == 0), stop=(i == 9))

            for q in range(NCHUNK):
                mm1(l_psum, 0, q)
            softmax_fin(0)
            for q in range(NCHUNK):
                mm2(y_psum, 0, q)
            for q in range(NCHUNK):
                mm1(l_psum, 1, q)
            softmax_fin(1)
            for q in range(NCHUNK):
                mm2(y_psum, 1, q)

    nc.finalize()
    return nc


def _get_nc() -> bass.Bass:
    global _cached_nc
    if _cached_nc is None:
        _cached_nc = _build_nc()
    return _cached_nc


def _fold_weights(w1, m0, m1, w2, gamma, beta, bn_mean, bn_var):
    w1 = np.asarray(w1, np.float64)
    m0 = np.asarray(m0, np.float64)
    m1 = np.asarray(m1, np.float64)
    w2 = np.asarray(w2, np.float64)
    gamma = np.asarray(gamma, np.float64)
    beta = np.asarray(beta, np.float64)
    bn_mean = np.asarray(bn_mean, np.float64)
    bn_var = np.asarray(bn_var, np.float64)

    wm_aug = np.zeros((C, K + 1), np.float32)
    wm_aug[:, :K] = (w1 @ m0).astype(np.float32)  # col K stays 0 -> ones row
    scale = gamma / np.sqrt(bn_var + BN_EPS)
    w2m_aug = np.zeros((K + 1, C), np.float32)
    w2m_aug[:K] = (m1 @ (w2 * scale[None, :])).astype(np.float32)
    w2m_aug[K] = (beta - bn_mean * scale).astype(np.float32)  # shift row
    return wm_aug, w2m_aug


def _run(inputs_np: dict, trace: bool = False):
    nc = _get_nc()
    inp = np.asarray(inputs_np["inputs"], np.float32).reshape(B, N, C)
    # transposed bf16 layout [B, C, N] so device DMAs are contiguous
    xt = inp.transpose(0, 2, 1).astype(NPBF16)
    wm_aug, w2m_aug = _fold_weights(
        inputs_np["w1"], inputs_np["m0"], inputs_np["m1"], inputs_np["w2"],
        inputs_np["gamma"], inputs_np["beta"],
        inputs_np["bn_mean"], inputs_np["bn_var"],
    )
    # pre-swizzle wm rows to [p, c4*k] so the const DMA is contiguous
    wm_sw = np.ascontiguousarray(
        wm_aug.reshape(4, 128, K + 1).transpose(1, 0, 2)
    ).reshape(128, 4 * (K + 1)).astype(NPBF16)
    w2m_bf = w2m_aug.astype(NPBF16)
    eye = np.eye(128, dtype=np.float32).astype(NPBF16)
    in_maps = [
        {
            "x": np.ascontiguousarray(xt[i * BPC:(i + 1) * BPC]),
            "wm": wm_sw,
            "w2m": w2m_bf,
            "ident": eye,
        }
        for i in range(NCORES)
    ]
    res = run_bass_kernel_spmd(nc, in_maps, core_ids=list(range(NCORES)),
                               trace=trace)
    out = np.concatenate([r["y"] for r in res.results], axis=0)  # [B, C, N]
    out = out.astype(np.float32).transpose(0, 2, 1).reshape(B, H, W, C)
    return np.ascontiguousarray(out), res


def kernel(**inputs) -> np.ndarray:
    out, _ = _run(inputs, trace=False)
    return out


# revision 4
# speedup vs baseline: 1.6018x; 1.2749x over previous
"""ExternalAttention Trainium2 Bass kernel (bf16 I/O, transposed layout).

Math (per batch b, with N = H*W = 4096 tokens, C = 512, K = 64):
    x      = inputs @ w1 + b1          [N, C]
    logits = x @ m0                    [N, K]
    attn   = softmax(logits, axis=N)   (the L1-normalize over N afterwards is a
                                        divide by 1 + 1e-9 -> skipped; the max
                                        subtraction is shift-invariant and
                                        logits are O(1) -> skipped)
    y      = attn @ m1 @ w2            [N, C]
    out    = relu(BN_affine(y) + inputs)

Host-side folds (all tiny C x C / C x K matrices):
    wm    = [w1 @ m0 | 0]                           [C, K+1]  (b1 @ m0 shifts each
            softmax column by a constant -> softmax-invariant, dropped; the zero
            column makes exp produce a ones-row that injects the BN shift)
    scale = gamma / sqrt(bn_var + eps); shift = beta - bn_mean * scale
    w2m   = [m1 @ (w2 * scale) ; shift]             [K+1, C]
    => out = relu(colsoftmax(inputs @ wm_aug) @ w2m_aug + inputs)

The kernel is PE/HBM-balanced, so everything is stored bf16 (rel-err budget
2e-2, bf16 contributes ~4e-3) and the host pre-transposes inputs to x^T
[C, N] per batch so no PE transposes / psum copies are needed on device.

Device kernel (per core, 2 batches, data-parallel over B=16 on 8 cores):
    - loads: c4-major [128, 2048] bf16 half-tiles on the sync HWDGE ring,
      each gating exactly one 4-matmul mm1 group
    - mm1 (per half): for c4 (weights loaded once): 4 chunk-matmuls
      accumulating logitsT [65, 512] psum chunks; ACT exp psum -> attn bf16
      with accumulated row sums
    - softmax normalization folded into w2m rows: w2m_s = w2m * (1/total)
      (65 x 512 DVE scale instead of 65 x 4096)
    - mm2 (per half, per c4): 4 start-matmuls w2m_s^T @ attn, then 4
      residual stop-matmuls I^T @ xT into the same psum tiles (weights
      loaded once per pass), relu psum -> bf16 in-place into the xT tiles
      (ACT / DVE groups alternate), store [128, 2048] on the sync ring
    - phase order b0mm1, b0mm2(h0), b1mm1, b0mm2(h1), b1mm2 keeps the PE
      busy across both softmax dependency chains
    - 4 zero-weight warmup matmuls at t0 (no DMA dependency) lift the PE
      HAM clock gate 1.2 -> 2.4 GHz before the first data lands
"""

import os
import sys
from contextlib import ExitStack

import numpy as np
import ml_dtypes

for _p in ("/opt/trn_rl_repo", os.path.expanduser("~/.axon_site/_ro/trn_rl_repo")):
    if os.path.isdir(_p) and _p not in sys.path:
        sys.path.insert(0, _p)

import concourse.bass as bass
import concourse.mybir as mybir
import concourse.tile as tile
from concourse import bacc
from concourse.bass import ts
from concourse.bass_utils import run_bass_kernel_spmd

B, H, W, C, K = 16, 64, 64, 512, 64
N = H * W  # 4096 tokens
BN_EPS = 1e-3
NCORES = 8
BPC = B // NCORES  # batches per core = 2
NCHUNK = 8  # 512-token chunks per batch

F32 = mybir.dt.float32
BF16 = mybir.dt.bfloat16
NPBF16 = ml_dtypes.bfloat16

_cached_nc = None


def _build_nc() -> bass.Bass:
    nc = bacc.Bacc(None, target_bir_lowering=False, debug=False)
    x = nc.dram_tensor("x", [BPC, C, N], BF16, kind="ExternalInput")
    wm = nc.dram_tensor("wm", [128, 4 * (K + 1)], BF16, kind="ExternalInput")
    w2m = nc.dram_tensor("w2m", [K + 1, C], BF16, kind="ExternalInput")
    ident = nc.dram_tensor("ident", [128, 128], BF16, kind="ExternalInput")
    y = nc.dram_tensor("y", [BPC, C, N], BF16, kind="ExternalOutput")

    with tile.TileContext(nc) as tc, ExitStack() as ctx:
        const = ctx.enter_context(tc.tile_pool(name="const", bufs=1))
        xt_pool = ctx.enter_context(tc.tile_pool(name="xt", bufs=2 * 4))
        attn_pool = ctx.enter_context(tc.tile_pool(name="attn", bufs=2))
        small = ctx.enter_context(tc.tile_pool(name="small", bufs=2))

        ident_sb = const.tile([128, 128], BF16)
        wm_sb = const.tile([128, 4, K + 1], BF16)  # [p, c4, k] = wm[c4*128+p, k]
        w2m_sb = const.tile([K + 1, C], BF16)
        zw_sb = const.tile([128, 128], BF16)   # zero warmup weights
        warm_sb = const.tile([128, 512], BF16)  # zero warmup rhs

        xv = [x[b].rearrange("(c4 p) n -> c4 p n", p=128) for b in range(BPC)]
        yv = [y[b].rearrange("(c4 p) n -> c4 p n", p=128) for b in range(BPC)]

        xts, attns, sums_t, w2ms = [], [], [], []
        for b in range(BPC):
            xts.append([
                xt_pool.tile([128, N], BF16, tag="xt", name=f"xt{b}_{c4}")
                for c4 in range(4)
            ])
            attns.append(attn_pool.tile([K + 1, N], BF16, tag="attn",
                                        name=f"attn{b}"))
            sums_t.append(small.tile([K + 1, NCHUNK], F32, tag="sums",
                                     name=f"sums{b}"))
            w2ms.append(small.tile([K + 1, C], BF16, tag="w2ms",
                                   name=f"w2ms{b}"))

        # wm leads (needed by the first mm1 group ~10us in); warmup weights
        # come from memsets so the PE can start with zero DMA dependency
        nc.sync.dma_start(
            out=wm_sb, in_=wm.rearrange("p (c4 k) -> p c4 k", c4=4))
        nc.vector.memset(zw_sb, 0.0)
        nc.vector.memset(warm_sb, 0.0)

        def load_half(b, h):
            hs = ts(h, N // 2)
            for c4 in range(4):
                nc.sync.dma_start(out=xts[b][c4][:, hs], in_=xv[b][c4][:, hs])

        load_half(0, 0)
        # consts needed from ~20us ride behind the first critical half
        nc.sync.dma_start(out=ident_sb, in_=ident[:, :])
        nc.sync.dma_start(out=w2m_sb, in_=w2m[:, :])
        load_half(0, 1)
        load_half(1, 0)
        load_half(1, 1)

        pool_ctx = tc.tile_pool(name="ps", bufs=8, space="PSUM")

        def mm1_half(ps, b, h):
            """logitsT psum chunks + exp for one 2048-token half."""
            qs = [4 * h + i for i in range(4)]
            tiles = [ps.tile([128, 512], F32, tag="ps", name=f"l{b}_{q}")
                     for q in qs]
            for c4 in range(4):
                for t, q in zip(tiles, qs):
                    nc.tensor.matmul(
                        t[0:K + 1],
                        lhsT=wm_sb[:, c4],
                        rhs=xts[b][c4][:, ts(q, 512)],
                        start=(c4 == 0),
                        stop=(c4 == 3),
                        skip_group_check=True,
                    )
            for t, q in zip(tiles, qs):
                # row K is exp(0)=1 (ones row); per-chunk row sums -> sums[:, q]
                nc.scalar.activation(
                    out=attns[b][:, ts(q, 512)], in_=t[0:K + 1],
                    func=mybir.ActivationFunctionType.Exp,
                    accum_out=sums_t[b][:, q:q + 1],
                )

        def chain(b):
            """Fold column-softmax normalization into the w2m rows."""
            total = small.tile([K + 1, 1], F32, tag="tot")
            nc.vector.reduce_sum(out=total, in_=sums_t[b],
                                 axis=mybir.AxisListType.X)
            rsum = small.tile([K + 1, 1], F32, tag="rs")
            nc.vector.reciprocal(out=rsum, in_=total)
            nc.vector.tensor_scalar_mul(w2ms[b][0:K], w2m_sb[0:K], rsum[0:K])
            # shift row (ones row of attn) stays unscaled
            nc.vector.tensor_copy(w2ms[b][K:K + 1], w2m_sb[K:K + 1])

        def mm2_group(ps, b, h, c4, engines):
            """One (half, c4) output group: 4 attn-matmuls + 4 residual
            matmuls into psum, relu drains, one [128, 2048] store."""
            qs = [4 * h + i for i in range(4)]
            tiles = [ps.tile([128, 512], F32, tag="ps", name=f"y{b}_{c4}_{q}")
                     for q in qs]
            for t, q in zip(tiles, qs):
                nc.tensor.matmul(t, lhsT=w2ms[b][:, ts(c4, 128)],
                                 rhs=attns[b][:, ts(q, 512)],
                                 start=True, stop=False, skip_group_check=True)
            for t, q in zip(tiles, qs):
                # residual: psum += I^T @ xT chunk
                nc.tensor.matmul(t, lhsT=ident_sb,
                                 rhs=xts[b][c4][:, ts(q, 512)],
                                 start=False, stop=True, skip_group_check=True)
            for t, q, eng in zip(tiles, qs, engines):
                if eng == "a":
                    nc.scalar.activation(
                        out=xts[b][c4][:, ts(q, 512)], in_=t,
                        func=mybir.ActivationFunctionType.Relu)
                else:
                    nc.vector.tensor_scalar_max(
                        xts[b][c4][:, ts(q, 512)], t, 0.0)
            nc.sync.dma_start(out=yv[b][c4][:, ts(h, N // 2)],
                              in_=xts[b][c4][:, ts(h, N // 2)])

        with pool_ctx as ps:
            # PE warmup on zero weights: lift the HAM clock gate while the
            # first loads land
            wp = ps.tile([128, 512], F32, tag="ps", name="warm")
            for i in range(4):
                nc.tensor.matmul(wp, lhsT=zw_sb, rhs=warm_sb,
                                 start=(i == 0), stop=(i == 3),
                                 skip_group_check=True)

            mm1_half(ps, 0, 0)
            mm1_half(ps, 0, 1)
            chain(0)
            for c4, engs in ((0, "aaaa"), (1, "vvvv"), (2, "aaaa"),
                             (3, "vvvv")):
                mm2_group(ps, 0, 0, c4, engs)
            mm1_half(ps, 1, 0)
            mm1_half(ps, 1, 1)
            chain(1)
            for c4, engs in ((0, "aaaa"), (1, "vvvv"), (2, "aaaa"),
                             (3, "vvvv")):
                mm2_group(ps, 0, 1, c4, engs)
            for c4, engs in ((0, "aaaa"), (1, "vvvv"), (2, "aaaa"),
                             (3, "vvvv")):
                mm2_group(ps, 1, 0, c4, engs)
            # tail half: alternate drain engines inside each group so the
            # final drains split across ACT+DVE (shorter store tail)
            for c4, engs in ((0, "aaaa"), (1, "vvvv"), (2, "avav"),
                             (3, "vava")):
                mm2_group(ps, 1, 1, c4, engs)

    nc.finalize()
    return nc


def _get_nc() -> bass.Bass:
    global _cached_nc
    if _cached_nc is None:
        _cached_nc = _build_nc()
    return _cached_nc


def _fold_weights(w1, m0, m1, w2, gamma, beta, bn_mean, bn_var):
    w1 = np.asarray(w1, np.float64)
    m0 = np.asarray(m0, np.float64)
    m1 = np.asarray(m1, np.float64)
    w2 = np.asarray(w2, np.float64)
    gamma = np.asarray(gamma, np.float64)
    beta = np.asarray(beta, np.float64)
    bn_mean = np.asarray(bn_mean, np.float64)
    bn_var = np.asarray(bn_var, np.float64)

    wm_aug = np.zeros((C, K + 1), np.float32)
    wm_aug[:, :K] = (w1 @ m0).astype(np.float32)  # col K stays 0 -> ones row
    scale = gamma / np.sqrt(bn_var + BN_EPS)
    w2m_aug = np.zeros((K + 1, C), np.float32)
    w2m_aug[:K] = (m1 @ (w2 * scale[None, :])).astype(np.float32)
    w2m_aug[K] = (beta - bn_mean * scale).astype(np.float32)  # shift row
    return wm_aug, w2m_aug


def _run(inputs_np: dict, trace: bool = False):
    nc = _get_nc()
    inp = np.asarray(inputs_np["inputs"], np.float32).reshape(B, N, C)
    # transposed bf16 layout [B, C, N] so device DMAs are contiguous
    xt = inp.transpose(0, 2, 1).astype(NPBF16)
    wm_aug, w2m_aug = _fold_weights(
        inputs_np["w1"], inputs_np["m0"], inputs_np["m1"], inputs_np["w2"],
        inputs_np["gamma"], inputs_np["beta"],
        inputs_np["bn_mean"], inputs_np["bn_var"],
    )
    # pre-swizzle wm rows to [p, c4*k] so the const DMA is contiguous
    wm_sw = np.ascontiguousarray(
        wm_aug.reshape(4, 128, K + 1).transpose(1, 0, 2)
    ).reshape(128, 4 * (K + 1)).astype(NPBF16)
    w2m_bf = w2m_aug.astype(NPBF16)
    eye = np.eye(128, dtype=np.float32).astype(NPBF16)
    in_maps = [
        {
            "x": np.ascontiguousarray(xt[i * BPC:(i + 1) * BPC]),
            "wm": wm_sw,
            "w2m": w2m_bf,
            "ident": eye,
        }
        for i in range(NCORES)
    ]
    res = run_bass_kernel_spmd(nc, in_maps, core_ids=list(range(NCORES)),
                               trace=trace)
    out = np.concatenate([r["y"] for r in res.results], axis=0)  # [B, C, N]
    out = out.astype(np.float32).transpose(0, 2, 1).reshape(B, H, W, C)
    return np.ascontiguousarray(out), res


def kernel(**inputs) -> np.ndarray:
    out, _ = _run(inputs, trace=False)
    return out


# revision 5
# speedup vs baseline: 1.6782x; 1.0477x over previous
"""ExternalAttention Trainium2 Bass kernel (bf16 I/O, transposed layout).

Math (per batch b, with N = H*W = 4096 tokens, C = 512, K = 64):
    x      = inputs @ w1 + b1          [N, C]
    logits = x @ m0                    [N, K]
    attn   = softmax(logits, axis=N)   (the L1-normalize over N afterwards is a
                                        divide by 1 + 1e-9 -> skipped; the max
                                        subtraction is shift-invariant and
                                        logits are O(1) -> skipped)
    y      = attn @ m1 @ w2            [N, C]
    out    = relu(BN_affine(y) + inputs)

Host-side folds (all tiny C x C / C x K matrices):
    wm    = [w1 @ m0 | 0]                           [C, K+1]  (b1 @ m0 shifts each
            softmax column by a constant -> softmax-invariant, dropped; the zero
            column makes exp produce a ones-row that injects the BN shift)
    scale = gamma / sqrt(bn_var + eps); shift = beta - bn_mean * scale
    w2m   = [m1 @ (w2 * scale) ; shift]             [K+1, C]
    => out = relu(colsoftmax(inputs @ wm_aug) @ w2m_aug + inputs)

The kernel is PE/HBM-balanced, so everything is stored bf16 (rel-err budget
2e-2, bf16 contributes ~4e-3) and the host pre-transposes inputs to x^T
[C, N] per batch so no PE transposes / psum copies are needed on device.

Device kernel (per core, 2 batches, data-parallel over B=16 on 8 cores):
    - loads: c4-major [128, 2048] bf16 half-tiles on the sync HWDGE ring,
      each gating exactly one 4-matmul mm1 group
    - mm1 (per half): for c4 (weights loaded once): 4 chunk-matmuls
      accumulating logitsT [65, 512] psum chunks; ACT exp psum -> attn bf16
      with accumulated row sums
    - softmax normalization folded into w2m rows: w2m_s = w2m * (1/total)
      (65 x 512 DVE scale instead of 65 x 4096)
    - mm2 (per half, per c4): 4 start-matmuls w2m_s^T @ attn, then 4
      residual stop-matmuls I^T @ xT into the same psum tiles (weights
      loaded once per pass), relu psum -> bf16 in-place into the xT tiles
      (ACT / DVE groups alternate), store [128, 2048] on the sync ring
    - phase order b0mm1, b0mm2(h0), b1mm1, b0mm2(h1), b1mm2 keeps the PE
      busy across both softmax dependency chains
    - 4 zero-weight warmup matmuls at t0 (no DMA dependency) lift the PE
      HAM clock gate 1.2 -> 2.4 GHz before the first data lands
"""

import os
import sys
from contextlib import ExitStack

import numpy as np
import ml_dtypes

for _p in ("/opt/trn_rl_repo", os.path.expanduser("~/.axon_site/_ro/trn_rl_repo")):
    if os.path.isdir(_p) and _p not in sys.path:
        sys.path.insert(0, _p)

import concourse.bass as bass
import concourse.mybir as mybir
import concourse.tile as tile
from concourse import bacc
from concourse.bass import ts
from concourse.bass_utils import run_bass_kernel_spmd

B, H, W, C, K = 16, 64, 64, 512, 64
N = H * W  # 4096 tokens
BN_EPS = 1e-3
NCORES = 8
BPC = B // NCORES  # batches per core = 2
NCHUNK = 8  # 512-token chunks per batch

F32 = mybir.dt.float32
BF16 = mybir.dt.bfloat16
NPBF16 = ml_dtypes.bfloat16

_cached_nc = None


def _build_nc() -> bass.Bass:
    nc = bacc.Bacc(None, target_bir_lowering=False, debug=False)
    x = nc.dram_tensor("x", [BPC, C, N], BF16, kind="ExternalInput")
    wm = nc.dram_tensor("wm", [128, 4 * (K + 1)], BF16, kind="ExternalInput")
    w2m = nc.dram_tensor("w2m", [K + 1, C], BF16, kind="ExternalInput")
    ident = nc.dram_tensor("ident", [128, 128], BF16, kind="ExternalInput")
    y = nc.dram_tensor("y", [BPC, C, N], BF16, kind="ExternalOutput")

    with tile.TileContext(nc) as tc, ExitStack() as ctx:
        const = ctx.enter_context(tc.tile_pool(name="const", bufs=1))
        xt_pool = ctx.enter_context(tc.tile_pool(name="xt", bufs=2 * 4))
        attn_pool = ctx.enter_context(tc.tile_pool(name="attn", bufs=2))
        small = ctx.enter_context(tc.tile_pool(name="small", bufs=2))

        ident_sb = const.tile([128, 128], BF16)
        wm_sb = const.tile([128, 4, K + 1], BF16)  # [p, c4, k] = wm[c4*128+p, k]
        w2m_sb = const.tile([K + 1, C], BF16)
        zw_sb = const.tile([128, 128], BF16)   # zero warmup weights
        warm_sb = const.tile([128, 512], BF16)  # zero warmup rhs

        xv = [x[b].rearrange("(c4 p) n -> c4 p n", p=128) for b in range(BPC)]
        yv = [y[b].rearrange("(c4 p) n -> c4 p n", p=128) for b in range(BPC)]

        xts, attns, sums_t, w2ms = [], [], [], []
        for b in range(BPC):
            xts.append([
                xt_pool.tile([128, N], BF16, tag="xt", name=f"xt{b}_{c4}")
                for c4 in range(4)
            ])
            attns.append(attn_pool.tile([K + 1, N], BF16, tag="attn",
                                        name=f"attn{b}"))
            sums_t.append(small.tile([K + 1, NCHUNK], F32, tag="sums",
                                     name=f"sums{b}"))
            w2ms.append(small.tile([K + 1, C], BF16, tag="w2ms",
                                   name=f"w2ms{b}"))

        # wm leads (needed by the first mm1 group ~10us in); warmup weights
        # come from memsets so the PE can start with zero DMA dependency
        nc.sync.dma_start(
            out=wm_sb, in_=wm.rearrange("p (c4 k) -> p c4 k", c4=4))
        nc.vector.memset(zw_sb, 0.0)
        nc.vector.memset(warm_sb, 0.0)

        def load_half(b, h):
            hs = ts(h, N // 2)
            for c4 in range(4):
                nc.sync.dma_start(out=xts[b][c4][:, hs], in_=xv[b][c4][:, hs])

        load_half(0, 0)
        # consts needed from ~20us ride behind the first critical half
        nc.sync.dma_start(out=ident_sb, in_=ident[:, :])
        nc.sync.dma_start(out=w2m_sb, in_=w2m[:, :])
        load_half(0, 1)
        load_half(1, 0)
        load_half(1, 1)

        pool_ctx = tc.tile_pool(name="ps", bufs=8, space="PSUM")

        def mm1_half(ps, b, h, q_outer=False):
            """logitsT psum chunks + exp for one 2048-token half.

            c4-outer: one weight load per c4, exps bunch at the end (fine
            when the next phase hides them). q_outer: per-chunk c4 loop so
            each exp overlaps the next chunk's matmuls (use on the half
            whose exps gate the softmax chain on the critical path)."""
            qs = [4 * h + i for i in range(4)]
            tiles = [ps.tile([128, 512], F32, tag="ps", name=f"l{b}_{q}")
                     for q in qs]

            def exp(t, q):
                # row K is exp(0)=1 (ones row); per-chunk row sums -> sums[:, q]
                nc.scalar.activation(
                    out=attns[b][:, ts(q, 512)], in_=t[0:K + 1],
                    func=mybir.ActivationFunctionType.Exp,
                    accum_out=sums_t[b][:, q:q + 1],
                )

            if q_outer:
                for t, q in zip(tiles, qs):
                    for c4 in range(4):
                        nc.tensor.matmul(
                            t[0:K + 1], lhsT=wm_sb[:, c4],
                            rhs=xts[b][c4][:, ts(q, 512)],
                            start=(c4 == 0), stop=(c4 == 3),
                            skip_group_check=True)
                    exp(t, q)
            else:
                for c4 in range(4):
                    for t, q in zip(tiles, qs):
                        nc.tensor.matmul(
                            t[0:K + 1], lhsT=wm_sb[:, c4],
                            rhs=xts[b][c4][:, ts(q, 512)],
                            start=(c4 == 0), stop=(c4 == 3),
                            skip_group_check=True)
                for t, q in zip(tiles, qs):
                    exp(t, q)

        def chain(b):
            """Fold column-softmax normalization into the w2m rows."""
            total = small.tile([K + 1, 1], F32, tag="tot")
            nc.vector.reduce_sum(out=total, in_=sums_t[b],
                                 axis=mybir.AxisListType.X)
            rsum = small.tile([K + 1, 1], F32, tag="rs")
            nc.vector.reciprocal(out=rsum, in_=total)
            nc.vector.tensor_scalar_mul(w2ms[b][0:K], w2m_sb[0:K], rsum[0:K])
            # shift row (ones row of attn) stays unscaled
            nc.vector.tensor_copy(w2ms[b][K:K + 1], w2m_sb[K:K + 1])

        def mm2_group(ps, b, h, c4, engines, fine_stores=False):
            """One (half, c4) output group: 4 attn-matmuls + 4 residual
            matmuls into psum, relu drains, one [128, 2048] store."""
            qs = [4 * h + i for i in range(4)]
            tiles = [ps.tile([128, 512], F32, tag="ps", name=f"y{b}_{c4}_{q}")
                     for q in qs]
            for t, q in zip(tiles, qs):
                # residual first: psum = I^T @ xT chunk (no chain dependency)
                nc.tensor.matmul(t, lhsT=ident_sb,
                                 rhs=xts[b][c4][:, ts(q, 512)],
                                 start=True, stop=False, skip_group_check=True)
            for t, q in zip(tiles, qs):
                nc.tensor.matmul(t, lhsT=w2ms[b][:, ts(c4, 128)],
                                 rhs=attns[b][:, ts(q, 512)],
                                 start=False, stop=True, skip_group_check=True)
            for t, q, eng in zip(tiles, qs, engines):
                if eng == "a":
                    nc.scalar.activation(
                        out=xts[b][c4][:, ts(q, 512)], in_=t,
                        func=mybir.ActivationFunctionType.Relu)
                else:
                    nc.vector.tensor_scalar_max(
                        xts[b][c4][:, ts(q, 512)], t, 0.0)
            if fine_stores:
                for qp in range(2):
                    cs = ts(2 * h + qp, N // 4)
                    nc.sync.dma_start(out=yv[b][c4][:, cs],
                                      in_=xts[b][c4][:, cs])
            else:
                nc.sync.dma_start(out=yv[b][c4][:, ts(h, N // 2)],
                                  in_=xts[b][c4][:, ts(h, N // 2)])

        with pool_ctx as ps:
            # PE warmup on zero weights: lift the HAM clock gate while the
            # first loads land
            wp = ps.tile([128, 512], F32, tag="ps", name="warm")
            for i in range(6):
                nc.tensor.matmul(wp, lhsT=zw_sb, rhs=warm_sb,
                                 start=(i == 0), stop=(i == 5),
                                 skip_group_check=True)

            mm1_half(ps, 0, 0)
            mm1_half(ps, 0, 1, q_outer=True)
            chain(0)
            for c4, engs in ((0, "aaaa"), (1, "vvvv"), (2, "aaaa"),
                             (3, "vvvv")):
                mm2_group(ps, 0, 0, c4, engs)
            mm1_half(ps, 1, 0)
            mm1_half(ps, 1, 1)
            chain(1)
            for c4, engs in ((0, "aaaa"), (1, "vvvv"), (2, "aaaa"),
                             (3, "vvvv")):
                mm2_group(ps, 0, 1, c4, engs)
            for c4, engs in ((0, "aaaa"), (1, "vvvv"), (2, "aaaa"),
                             (3, "vvvv")):
                mm2_group(ps, 1, 0, c4, engs)
            # tail half: alternate drain engines inside each group so the
            # final drains split across ACT+DVE (shorter store tail)
            for c4, engs in ((0, "aaaa"), (1, "vvvv"), (2, "avav"),
                             (3, "vava")):
                mm2_group(ps, 1, 1, c4, engs, fine_stores=(c4 >= 2))

    nc.finalize()
    return nc


def _get_nc() -> bass.Bass:
    global _cached_nc
    if _cached_nc is None:
        _cached_nc = _build_nc()
    return _cached_nc


def _fold_weights(w1, m0, m1, w2, gamma, beta, bn_mean, bn_var):
    w1 = np.asarray(w1, np.float64)
    m0 = np.asarray(m0, np.float64)
    m1 = np.asarray(m1, np.float64)
    w2 = np.asarray(w2, np.float64)
    gamma = np.asarray(gamma, np.float64)
    beta = np.asarray(beta, np.float64)
    bn_mean = np.asarray(bn_mean, np.float64)
    bn_var = np.asarray(bn_var, np.float64)

    wm_aug = np.zeros((C, K + 1), np.float32)
    wm_aug[:, :K] = (w1 @ m0).astype(np.float32)  # col K stays 0 -> ones row
    scale = gamma / np.sqrt(bn_var + BN_EPS)
    w2m_aug = np.zeros((K + 1, C), np.float32)
    w2m_aug[:K] = (m1 @ (w2 * scale[None, :])).astype(np.float32)
    w2m_aug[K] = (beta - bn_mean * scale).astype(np.float32)  # shift row
    return wm_aug, w2m_aug


def _run(inputs_np: dict, trace: bool = False):
    nc = _get_nc()
    inp = np.asarray(inputs_np["inputs"], np.float32).reshape(B, N, C)
    # transposed bf16 layout [B, C, N] so device DMAs are contiguous
    xt = inp.transpose(0, 2, 1).astype(NPBF16)
    wm_aug, w2m_aug = _fold_weights(
        inputs_np["w1"], inputs_np["m0"], inputs_np["m1"], inputs_np["w2"],
        inputs_np["gamma"], inputs_np["beta"],
        inputs_np["bn_mean"], inputs_np["bn_var"],
    )
    # pre-swizzle wm rows to [p, c4*k] so the const DMA is contiguous
    wm_sw = np.ascontiguousarray(
        wm_aug.reshape(4, 128, K + 1).transpose(1, 0, 2)
    ).reshape(128, 4 * (K + 1)).astype(NPBF16)
    w2m_bf = w2m_aug.astype(NPBF16)
    eye = np.eye(128, dtype=np.float32).astype(NPBF16)
    in_maps = [
        {
            "x": np.ascontiguousarray(xt[i * BPC:(i + 1) * BPC]),
            "wm": wm_sw,
            "w2m": w2m_bf,
            "ident": eye,
        }
        for i in range(NCORES)
    ]
    res = run_bass_kernel_spmd(nc, in_maps, core_ids=list(range(NCORES)),
                               trace=trace)
    out = np.concatenate([r["y"] for r in res.results], axis=0)  # [B, C, N]
    out = out.astype(np.float32).transpose(0, 2, 1).reshape(B, H, W, C)
    return np.ascontiguousarray(out), res


def kernel(**inputs) -> np.ndarray:
    out, _ = _run(inputs, trace=False)
    return out


# revision 6
# speedup vs baseline: 1.6911x; 1.0077x over previous
"""ExternalAttention Trainium2 Bass kernel (bf16 I/O, transposed layout).

Math (per batch b, with N = H*W = 4096 tokens, C = 512, K = 64):
    x      = inputs @ w1 + b1          [N, C]
    logits = x @ m0                    [N, K]
    attn   = softmax(logits, axis=N)   (the L1-normalize over N afterwards is a
                                        divide by 1 + 1e-9 -> skipped; the max
                                        subtraction is shift-invariant and
                                        logits are O(1) -> skipped)
    y      = attn @ m1 @ w2            [N, C]
    out    = relu(BN_affine(y) + inputs)

Host-side folds (all tiny C x C / C x K matrices):
    wm    = [w1 @ m0 | 0]                           [C, K+1]  (b1 @ m0 shifts each
            softmax column by a constant -> softmax-invariant, dropped; the zero
            column makes exp produce a ones-row that injects the BN shift)
    scale = gamma / sqrt(bn_var + eps); shift = beta - bn_mean * scale
    w2m   = [m1 @ (w2 * scale) ; shift]             [K+1, C]
    => out = relu(colsoftmax(inputs @ wm_aug) @ w2m_aug + inputs)

The kernel is PE/HBM-balanced, so everything is stored bf16 (rel-err budget
2e-2, bf16 contributes ~4e-3) and the host pre-transposes inputs to x^T
[C, N] per batch so no PE transposes / psum copies are needed on device.

Device kernel (per core, 2 batches, data-parallel over B=16 on 8 cores):
    - loads: c4-major [128, 2048] bf16 half-tiles on the sync HWDGE ring,
      each gating exactly one 4-matmul mm1 group
    - mm1 (per half): for c4 (weights loaded once): 4 chunk-matmuls
      accumulating logitsT [65, 512] psum chunks; ACT exp psum -> attn bf16
      with accumulated row sums
    - softmax normalization folded into w2m rows: w2m_s = w2m * (1/total)
      (65 x 512 DVE scale instead of 65 x 4096)
    - mm2 (per half, per c4): 4 start-matmuls w2m_s^T @ attn, then 4
      residual stop-matmuls I^T @ xT into the same psum tiles (weights
      loaded once per pass), relu psum -> bf16 in-place into the xT tiles
      (ACT / DVE groups alternate), store [128, 2048] on the sync ring
    - phase order b0mm1, b0mm2(h0), b1mm1, b0mm2(h1), b1mm2 keeps the PE
      busy across both softmax dependency chains
    - 4 zero-weight warmup matmuls at t0 (no DMA dependency) lift the PE
      HAM clock gate 1.2 -> 2.4 GHz before the first data lands
"""

import os
import sys
from contextlib import ExitStack

import numpy as np
import ml_dtypes

for _p in ("/opt/trn_rl_repo", os.path.expanduser("~/.axon_site/_ro/trn_rl_repo")):
    if os.path.isdir(_p) and _p not in sys.path:
        sys.path.insert(0, _p)

import concourse.bass as bass
import concourse.mybir as mybir
import concourse.tile as tile
from concourse import bacc
from concourse.bass import ts
from concourse.bass_utils import run_bass_kernel_spmd

B, H, W, C, K = 16, 64, 64, 512, 64
N = H * W  # 4096 tokens
BN_EPS = 1e-3
NCORES = 8
BPC = B // NCORES  # batches per core = 2
NCHUNK = 8  # 512-token chunks per batch

F32 = mybir.dt.float32
BF16 = mybir.dt.bfloat16
NPBF16 = ml_dtypes.bfloat16

_cached_nc = None


def _build_nc() -> bass.Bass:
    nc = bacc.Bacc(None, target_bir_lowering=False, debug=False)
    x = nc.dram_tensor("x", [BPC, C, N], BF16, kind="ExternalInput")
    wm = nc.dram_tensor("wm", [128, 4 * (K + 1)], BF16, kind="ExternalInput")
    w2m = nc.dram_tensor("w2m", [K + 1, C], BF16, kind="ExternalInput")
    ident = nc.dram_tensor("ident", [128, 128], BF16, kind="ExternalInput")
    y = nc.dram_tensor("y", [BPC, C, N], BF16, kind="ExternalOutput")

    with tile.TileContext(nc) as tc, ExitStack() as ctx:
        const = ctx.enter_context(tc.tile_pool(name="const", bufs=1))
        xt_pool = ctx.enter_context(tc.tile_pool(name="xt", bufs=2 * 4))
        attn_pool = ctx.enter_context(tc.tile_pool(name="attn", bufs=2))
        small = ctx.enter_context(tc.tile_pool(name="small", bufs=2))

        ident_sb = const.tile([128, 128], BF16)
        wm_sb = const.tile([128, 4, K + 1], BF16)  # [p, c4, k] = wm[c4*128+p, k]
        w2m_sb = const.tile([K + 1, C], BF16)
        zw_sb = const.tile([128, 128], BF16)   # zero warmup weights
        warm_sb = const.tile([128, 512], BF16)  # zero warmup rhs

        xv = [x[b].rearrange("(c4 p) n -> c4 p n", p=128) for b in range(BPC)]
        yv = [y[b].rearrange("(c4 p) n -> c4 p n", p=128) for b in range(BPC)]

        xts, attns, sums_t, w2ms = [], [], [], []
        for b in range(BPC):
            xts.append([
                xt_pool.tile([128, N], BF16, tag="xt", name=f"xt{b}_{c4}")
                for c4 in range(4)
            ])
            attns.append(attn_pool.tile([K + 1, N], BF16, tag="attn",
                                        name=f"attn{b}"))
            sums_t.append(small.tile([K + 1, NCHUNK], F32, tag="sums",
                                     name=f"sums{b}"))
            w2ms.append(small.tile([K + 1, C], BF16, tag="w2ms",
                                   name=f"w2ms{b}"))

        # warmup weights come from memsets so the PE can start with zero
        # DMA dependency; the first critical x half-tile leads the ring and
        # the tiny wm rides just behind it
        nc.vector.memset(zw_sb, 0.0)
        nc.vector.memset(warm_sb, 0.0)

        def load_half(b, h, c4s=range(4)):
            hs = ts(h, N // 2)
            for c4 in c4s:
                nc.sync.dma_start(out=xts[b][c4][:, hs], in_=xv[b][c4][:, hs])

        load_half(0, 0, c4s=[0])
        nc.sync.dma_start(
            out=wm_sb, in_=wm.rearrange("p (c4 k) -> p c4 k", c4=4))
        load_half(0, 0, c4s=[1, 2, 3])
        # consts needed from ~20us ride behind the first critical half
        nc.sync.dma_start(out=ident_sb, in_=ident[:, :])
        nc.sync.dma_start(out=w2m_sb, in_=w2m[:, :])
        load_half(0, 1)
        load_half(1, 0)
        load_half(1, 1)

        pool_ctx = tc.tile_pool(name="ps", bufs=8, space="PSUM")

        def mm1_half(ps, b, h, q_outer=False):
            """logitsT psum chunks + exp for one 2048-token half.

            c4-outer: one weight load per c4, exps bunch at the end (fine
            when the next phase hides them). q_outer: per-chunk c4 loop so
            each exp overlaps the next chunk's matmuls (use on the half
            whose exps gate the softmax chain on the critical path)."""
            qs = [4 * h + i for i in range(4)]
            tiles = [ps.tile([128, 512], F32, tag="ps", name=f"l{b}_{q}")
                     for q in qs]

            def exp(t, q):
                # row K is exp(0)=1 (ones row); per-chunk row sums -> sums[:, q]
                nc.scalar.activation(
                    out=attns[b][:, ts(q, 512)], in_=t[0:K + 1],
                    func=mybir.ActivationFunctionType.Exp,
                    accum_out=sums_t[b][:, q:q + 1],
                )

            if q_outer:
                for t, q in zip(tiles, qs):
                    for c4 in range(4):
                        nc.tensor.matmul(
                            t[0:K + 1], lhsT=wm_sb[:, c4],
                            rhs=xts[b][c4][:, ts(q, 512)],
                            start=(c4 == 0), stop=(c4 == 3),
                            skip_group_check=True)
                    exp(t, q)
            else:
                for c4 in range(4):
                    for t, q in zip(tiles, qs):
                        nc.tensor.matmul(
                            t[0:K + 1], lhsT=wm_sb[:, c4],
                            rhs=xts[b][c4][:, ts(q, 512)],
                            start=(c4 == 0), stop=(c4 == 3),
                            skip_group_check=True)
                for t, q in zip(tiles, qs):
                    exp(t, q)

        def chain(b):
            """Fold column-softmax normalization into the w2m rows."""
            total = small.tile([K + 1, 1], F32, tag="tot")
            nc.vector.reduce_sum(out=total, in_=sums_t[b],
                                 axis=mybir.AxisListType.X)
            rsum = small.tile([K + 1, 1], F32, tag="rs")
            nc.vector.reciprocal(out=rsum, in_=total)
            nc.vector.tensor_scalar_mul(w2ms[b][0:K], w2m_sb[0:K], rsum[0:K])
            # shift row (ones row of attn) stays unscaled
            nc.vector.tensor_copy(w2ms[b][K:K + 1], w2m_sb[K:K + 1])

        def mm2_group(ps, b, h, c4, engines, fine_stores=False):
            """One (half, c4) output group: 4 attn-matmuls + 4 residual
            matmuls into psum, relu drains, one [128, 2048] store."""
            qs = [4 * h + i for i in range(4)]
            tiles = [ps.tile([128, 512], F32, tag="ps", name=f"y{b}_{c4}_{q}")
                     for q in qs]
            for t, q in zip(tiles, qs):
                # residual first: psum = I^T @ xT chunk (no chain dependency)
                nc.tensor.matmul(t, lhsT=ident_sb,
                                 rhs=xts[b][c4][:, ts(q, 512)],
                                 start=True, stop=False, skip_group_check=True)
            for t, q in zip(tiles, qs):
                nc.tensor.matmul(t, lhsT=w2ms[b][:, ts(c4, 128)],
                                 rhs=attns[b][:, ts(q, 512)],
                                 start=False, stop=True, skip_group_check=True)
            for t, q, eng in zip(tiles, qs, engines):
                if eng == "a":
                    nc.scalar.activation(
                        out=xts[b][c4][:, ts(q, 512)], in_=t,
                        func=mybir.ActivationFunctionType.Relu)
                else:
                    nc.vector.tensor_scalar_max(
                        xts[b][c4][:, ts(q, 512)], t, 0.0)
            if fine_stores:
                for q in qs:
                    nc.sync.dma_start(out=yv[b][c4][:, ts(q, 512)],
                                      in_=xts[b][c4][:, ts(q, 512)])
            else:
                nc.sync.dma_start(out=yv[b][c4][:, ts(h, N // 2)],
                                  in_=xts[b][c4][:, ts(h, N // 2)])

        with pool_ctx as ps:
            # PE warmup on zero weights: lift the HAM clock gate while the
            # first loads land
            wp = ps.tile([128, 512], F32, tag="ps", name="warm")
            for i in range(12):
                nc.tensor.matmul(wp, lhsT=zw_sb, rhs=warm_sb,
                                 start=(i == 0), stop=(i == 11),
                                 skip_group_check=True)

            mm1_half(ps, 0, 0)
            mm1_half(ps, 0, 1, q_outer=True)
            chain(0)
            for c4, engs in ((0, "aaaa"), (1, "vvvv"), (2, "aaaa"),
                             (3, "vvvv")):
                mm2_group(ps, 0, 0, c4, engs)
            mm1_half(ps, 1, 0)
            mm1_half(ps, 1, 1)
            chain(1)
            for c4, engs in ((0, "aaaa"), (1, "vvvv"), (2, "aaaa"),
                             (3, "vvvv")):
                mm2_group(ps, 0, 1, c4, engs)
            for c4, engs in ((0, "aaaa"), (1, "vvvv"), (2, "aaaa"),
                             (3, "vvvv")):
                mm2_group(ps, 1, 0, c4, engs)
            # tail half: alternate drain engines inside each group so the
            # final drains split across ACT+DVE (shorter store tail)
            for c4, engs in ((0, "aaaa"), (1, "vvvv"), (2, "avav"),
                             (3, "vava")):
                mm2_group(ps, 1, 1, c4, engs, fine_stores=(c4 == 3))

    nc.finalize()
    return nc


def _get_nc() -> bass.Bass:
    global _cached_nc
    if _cached_nc is None:
        _cached_nc = _build_nc()
    return _cached_nc


def _fold_weights(w1, m0, m1, w2, gamma, beta, bn_mean, bn_var):
    w1 = np.asarray(w1, np.float64)
    m0 = np.asarray(m0, np.float64)
    m1 = np.asarray(m1, np.float64)
    w2 = np.asarray(w2, np.float64)
    gamma = np.asarray(gamma, np.float64)
    beta = np.asarray(beta, np.float64)
    bn_mean = np.asarray(bn_mean, np.float64)
    bn_var = np.asarray(bn_var, np.float64)

    wm_aug = np.zeros((C, K + 1), np.float32)
    wm_aug[:, :K] = (w1 @ m0).astype(np.float32)  # col K stays 0 -> ones row
    scale = gamma / np.sqrt(bn_var + BN_EPS)
    w2m_aug = np.zeros((K + 1, C), np.float32)
    w2m_aug[:K] = (m1 @ (w2 * scale[None, :])).astype(np.float32)
    w2m_aug[K] = (beta - bn_mean * scale).astype(np.float32)  # shift row
    return wm_aug, w2m_aug


def _run(inputs_np: dict, trace: bool = False):
    nc = _get_nc()
    inp = np.asarray(inputs_np["inputs"], np.float32).reshape(B, N, C)
    # transposed bf16 layout [B, C, N] so device DMAs are contiguous
    xt = inp.transpose(0, 2, 1).astype(NPBF16)
    wm_aug, w2m_aug = _fold_weights(
        inputs_np["w1"], inputs_np["m0"], inputs_np["m1"], inputs_np["w2"],
        inputs_np["gamma"], inputs_np["beta"],
        inputs_np["bn_mean"], inputs_np["bn_var"],
    )
    # pre-swizzle wm rows to [p, c4*k] so the const DMA is contiguous
    wm_sw = np.ascontiguousarray(
        wm_aug.reshape(4, 128, K + 1).transpose(1, 0, 2)
    ).reshape(128, 4 * (K + 1)).astype(NPBF16)
    w2m_bf = w2m_aug.astype(NPBF16)
    eye = np.eye(128, dtype=np.float32).astype(NPBF16)
    in_maps = [
        {
            "x": np.ascontiguousarray(xt[i * BPC:(i + 1) * BPC]),
            "wm": wm_sw,
            "w2m": w2m_bf,
            "ident": eye,
        }
        for i in range(NCORES)
    ]
    res = run_bass_kernel_spmd(nc, in_maps, core_ids=list(range(NCORES)),
                               trace=trace)
    out = np.concatenate([r["y"] for r in res.results], axis=0)  # [B, C, N]
    out = out.astype(np.float32).transpose(0, 2, 1).reshape(B, H, W, C)
    return np.ascontiguousarray(out), res


def kernel(**inputs) -> np.ndarray:
    out, _ = _run(inputs, trace=False)
    return out
